# revision 2
# baseline (speedup 1.0000x reference)
"""DigitCaps routing kernel v2 for 8 Trainium2 NeuronCores.

Same I-sharding as v1 (144 input capsules per core, one AllReduce of the
s-partials per iteration, final ReduceScatter), but restructured to cut the
serial chain that dominates the runtime:

- fp16 collective payload (halves AR wire time; CCE adds in fp16).
- bf16 matmul operands (FWL halves weight-load time; PSUM stays fp32).
- squash rewritten as v = s*sqrt(u)/(den^2+u), u = sum_d s^2: 5 ops deep.
- softmax denominator replicated across all 128 payload partitions BEFORE
  the AllReduce, so the post-AR path needs no PE broadcast round trip;
  iteration 0 uses the compile-time constant den = I.
- b_ij is accumulated directly in PSUM across iterations by the agreement
  matmul (start=False on later iterations); e = exp(psum) on ACT.
- agreement G-tiles grouped 3-per-PSUM-bank: 3 wide TT+reduce pairs.
- dummy PE matmuls during each AR window keep the HAM clock at 2.4 GHz.
"""
import numpy as np

import concourse.bacc as bacc
import concourse.mybir as mybir
import concourse.tile as tile
from concourse.bass_utils import run_bass_kernel_spmd

N_CORES = 8
B, I, O, D, J = 512, 1152, 10, 16, 8
IL = I // N_CORES          # 144 local input capsules
G = IL * J // 128          # 9 ij tiles of 128 partitions
M = B // 128               # 4 batch chunks
OD = O * D                 # 160
GO = G * O                 # 90
PAY = M * OD + 16          # payload width with den/pad cols
NIT = 3
F32 = mybir.dt.float32
F16 = mybir.dt.float16
BF16 = mybir.dt.bfloat16
Act = mybir.ActivationFunctionType
Alu = mybir.AluOpType
DEN0 = float(I)            # iteration-0 softmax denominator (b=0 -> e=1)

_cache = {}


def _build(repeat=1, no_ar=False, warm=16):
    nc = bacc.Bacc("TRN2", target_bir_lowering=False, debug=False,
                   num_devices=N_CORES)
    xT_e = nc.dram_tensor("xT", [IL * J, B], BF16, kind="ExternalInput")
    xN_e = nc.dram_tensor("xN", [B, IL * J], BF16, kind="ExternalInput")
    w2_e = nc.dram_tensor("w2", [IL * J, OD], BF16, kind="ExternalInput")
    ind_e = nc.dram_tensor("ind", [16, 128], BF16, kind="ExternalInput")
    indj_e = nc.dram_tensor("indj", [128, 16], F32, kind="ExternalInput")
    v_e = nc.dram_tensor("v_out", [16, M * OD], F32, kind="ExternalOutput")

    with tile.TileContext(nc) as tc:
        with (
            tc.tile_pool(name="const", bufs=1) as constp,
            tc.tile_pool(name="big", bufs=1) as big,
            tc.tile_pool(name="wcp", bufs=2) as wcp,
            tc.tile_pool(name="work", bufs=2) as work,
            tc.tile_pool(name="ps_s", bufs=2, space="PSUM") as ps_s_pool,
            tc.tile_pool(name="ps_g", bufs=2, space="PSUM") as ps_g_pool,
            tc.tile_pool(name="ps_c", bufs=2, space="PSUM") as ps_c_pool,
            tc.tile_pool(name="ps_w", bufs=1, space="PSUM") as ps_w_pool,
            tc.tile_pool(name="ps_m", bufs=1, space="PSUM") as ps_m_pool,
            tc.tile_pool(name="dram", bufs=2, space="DRAM") as dram,
        ):
            # ---- persistent inputs ----
            xT = big.tile([128, G * B], BF16)       # [p=(i16,j8), (g, b)]
            w2 = big.tile([128, G * OD], BF16)      # [p=(i16,j8), (g, o, d)]
            xN = big.tile([128, M * IL * J], BF16)  # [p=b, (m, ij)]
            for g in range(G):
                nc.sync.dma_start(out=w2[:, g * OD:(g + 1) * OD],
                                  in_=w2_e[g * 128:(g + 1) * 128, :])
                nc.sync.dma_start(out=xT[:, g * B:(g + 1) * B],
                                  in_=xT_e[g * 128:(g + 1) * 128, :])
            for m in range(M):
                nc.sync.dma_start(out=xN[:, m * IL * J:(m + 1) * IL * J],
                                  in_=xN_e[m * 128:(m + 1) * 128, :])
            ind = constp.tile([16, 128], BF16)
            nc.sync.dma_start(out=ind[:], in_=ind_e[:])
            indj = constp.tile([128, 16], F32)
            nc.sync.dma_start(out=indj[:], in_=indj_e[:])
            ones1 = constp.tile([1, 128], F32)
            nc.vector.memset(ones1[:], 1.0)
            ones16 = constp.tile([16, 1], BF16)
            nc.vector.memset(ones16[:], 1.0)

            for rep in range(repeat):
              # one PSUM bank shared by the small odds and ends:
              # [0:16, 0:90] b accumulation; [0:1, 96:186] den partial;
              # [:, 192:208] den broadcast
              ps_m = ps_m_pool.tile([128, 512], F32, name=f"ps_m{rep}",
                                    tag="psm")
              ps_b = ps_m[0:16, 0:GO]
              b_sb = None
              for t in range(NIT):
                last = t == NIT - 1
                if t > 0:
                    # e = exp(b); b accumulated in SBUF
                    e_sb = work.tile([16, GO], BF16)
                    nc.scalar.activation(e_sb[:], b_sb[:], Act.Exp)
                    # den[o] = sum_i e[i,o]: PE over i16, DVE over g
                    ps_d1 = ps_m[0:1, 96:96 + GO]
                    nc.tensor.matmul(ps_d1, ones16[:], e_sb[:],
                                     start=True, stop=True)
                    denrow = work.tile([1, 16], F32, name="denrow")
                    nc.vector.memset(denrow[:, O:], 0.0)
                    nc.vector.reduce_sum(
                        denrow[0:1, 0:O],
                        ps_d1.rearrange("p (g o) -> p o g", g=G),
                        axis=mybir.AxisListType.X)
                    # Wc = w2 * broadcast(e): 3 wide bc-MMs + 3 wide TTs
                    wc = wcp.tile([128, G * OD], BF16)
                    for c in range(3):
                        ps_ce = ps_c_pool.tile([128, 3 * OD], F32)
                        rhs = e_sb[:, c * 30:(c + 1) * 30] \
                            .rearrange("p (g o) -> p g o", g=3) \
                            .unsqueeze(3).broadcast_to([16, 3, O, D])
                        nc.tensor.matmul(ps_ce[:], ind[:], rhs,
                                         start=True, stop=True)
                        nc.vector.tensor_tensor(
                            wc[:, c * 3 * OD:(c + 1) * 3 * OD],
                            w2[:, c * 3 * OD:(c + 1) * 3 * OD], ps_ce[:],
                            op=Alu.mult)
                    s_rhs = wc
                else:
                    s_rhs = w2

                # partial s over the 9 local ij tiles; payload [128, PAY] fp16
                # (t=0 payload is only M*OD wide: den is the constant I).
                pw = M * OD if t == 0 else PAY
                ar_in = dram.tile([128, pw], F16, name="ar_in")
                ar_out = dram.tile([128, pw], F16, name="ar_out")
                s_stage = work.tile([128, pw], F16)
                if t > 0:
                    # replicate den over all 128 partitions pre-collective:
                    # the element-wise AR/RS sum then delivers the full den
                    # to every partition with no post-AR broadcast needed.
                    ps_db = ps_m[:, 192:208]
                    nc.tensor.matmul(ps_db, ones1[:], denrow[:],
                                     start=True, stop=True)
                    nc.vector.tensor_copy(s_stage[:, M * OD:PAY], ps_db)
                for m in range(M):
                    ps_s = ps_s_pool.tile([128, OD], F32)
                    for g in range(G):
                        nc.tensor.matmul(
                            ps_s[:],
                            xT[:, g * B + m * 128: g * B + (m + 1) * 128],
                            s_rhs[:, g * OD:(g + 1) * OD],
                            start=(g == 0), stop=(g == G - 1))
                    nc.vector.tensor_copy(s_stage[:, m * OD:(m + 1) * OD],
                                          ps_s[:])
                    if m == 1:
                        nc.sync.dma_start(out=ar_in[:, 0:2 * OD],
                                          in_=s_stage[:, 0:2 * OD])
                    elif m == M - 1:
                        nc.sync.dma_start(out=ar_in[:, 2 * OD:pw],
                                          in_=s_stage[:, 2 * OD:pw])

                if last:
                    rs_out = dram.tile([16, pw], F16, name="rs_out")
                    if no_ar:
                        nc.sync.dma_start(out=rs_out[:, :], in_=ar_in[0:16, :])
                    else:
                        nc.gpsimd.collective_compute(
                            "ReduceScatter", Alu.add,
                            replica_groups=[list(range(N_CORES))],
                            ins=[ar_in.opt()], outs=[rs_out.opt()])
                    sl = work.tile([16, pw], F16)
                    nc.sync.dma_start(out=sl[:, :], in_=rs_out[:, :])
                    # squash on the 16-partition slice
                    sq16 = work.tile([16, M * OD], BF16)
                    nc.vector.tensor_tensor(sq16[:], sl[:, 0:M * OD],
                                            sl[:, 0:M * OD], op=Alu.mult)
                    u16 = work.tile([16, M * O], F32)
                    nc.vector.reduce_sum(
                        u16[:],
                        sq16[:].rearrange("p (m o d) -> p m o d", m=M, o=O),
                        axis=mybir.AxisListType.X)
                    dn16 = work.tile([16, O], F32)
                    nc.scalar.activation(dn16[:], sl[:, M * OD:M * OD + O],
                                         Act.Square)
                    t216 = work.tile([16, M * O], F32)
                    nc.vector.tensor_tensor(
                        t216[:].rearrange("p (m o) -> p m o", m=M),
                        u16[:].rearrange("p (m o) -> p m o", m=M),
                        dn16[:].unsqueeze(1).broadcast_to([16, M, O]),
                        op=Alu.add)
                    rt16 = work.tile([16, M * O], F32)
                    nc.scalar.activation(rt16[:], u16[:], Act.Sqrt)
                    rc16 = work.tile([16, M * O], F32)
                    nc.vector.reciprocal(rc16[:], t216[:])
                    gf16 = work.tile([16, M * O], F32)
                    nc.vector.tensor_tensor(gf16[:], rt16[:], rc16[:],
                                            op=Alu.mult)
                    vsl = work.tile([16, M * OD], F32)
                    nc.vector.tensor_tensor(
                        vsl[:].rearrange("p (m o d) -> p m o d", m=M, o=O),
                        sl[:, 0:M * OD].rearrange("p (m o d) -> p m o d",
                                                  m=M, o=O),
                        gf16[:].rearrange("p (m o) -> p m o", m=M)
                        .unsqueeze(3).broadcast_to([16, M, O, D]),
                        op=Alu.mult)
                    nc.sync.dma_start(out=v_e[:, :], in_=vsl[:])
                    continue

                if no_ar:
                    nc.sync.dma_start(out=ar_out[:, :], in_=ar_in[:, :])
                else:
                    nc.gpsimd.collective_compute(
                        "AllReduce", Alu.add,
                        replica_groups=[list(range(N_CORES))],
                        ins=[ar_in.opt()], outs=[ar_out.opt()])

                if warm:
                    # junk matmuls keep the PE HAM clock warm through the AR
                    ps_j = ps_w_pool.tile([128, 512], F32, name="ps_j",
                                          tag="warm")
                    for w in range(warm):
                        nc.tensor.matmul(ps_j[:], xT[:, 0:128], xT[:, 0:512],
                                         start=(w == 0), stop=(w == warm - 1))

                s_sb = work.tile([128, pw], F16)
                nc.sync.dma_start(out=s_sb[:, :], in_=ar_out[:, :])
                # squash: v = s*sqrt(u)/(den^2+u), u = sum_d s^2
                sqr = work.tile([128, M * OD], BF16)
                nc.vector.tensor_tensor(sqr[:], s_sb[:, 0:M * OD],
                                        s_sb[:, 0:M * OD], op=Alu.mult)
                u = work.tile([128, M * O], F32)
                nc.vector.reduce_sum(
                    u[:], sqr[:].rearrange("p (m o d) -> p m o d", m=M, o=O),
                    axis=mybir.AxisListType.X)
                rt = work.tile([128, M * O], F32)
                nc.scalar.activation(rt[:], u[:], Act.Sqrt)
                t2 = work.tile([128, M * O], F32)
                if t == 0:
                    # den is the constant I: gf = rt/(u + I^2)
                    nc.vector.tensor_scalar_add(t2[:], u[:], DEN0 * DEN0)
                else:
                    den2 = work.tile([128, O], F32)
                    nc.scalar.activation(den2[:], s_sb[:, M * OD:M * OD + O],
                                         Act.Square)
                    nc.vector.tensor_tensor(
                        t2[:].rearrange("p (m o) -> p m o", m=M),
                        u[:].rearrange("p (m o) -> p m o", m=M),
                        den2[:].unsqueeze(1).broadcast_to([128, M, O]),
                        op=Alu.add)
                rc = work.tile([128, M * O], F32)
                nc.vector.reciprocal(rc[:], t2[:])
                gf = work.tile([128, M * O], F32)
                nc.vector.tensor_tensor(gf[:], rt[:], rc[:], op=Alu.mult)
                v_sb = work.tile([128, M * OD], BF16)
                nc.vector.tensor_tensor(
                    v_sb[:].rearrange("p (m o d) -> p m o d", m=M, o=O),
                    s_sb[:, 0:M * OD].rearrange("p (m o d) -> p m o d",
                                                m=M, o=O),
                    gf[:].rearrange("p (m o) -> p m o", m=M).unsqueeze(3)
                    .broadcast_to([128, M, O, D]),
                    op=Alu.mult)

                # agreement: G = sum_b x (x) v, 3 g-tiles per PSUM bank;
                # P4 = w2*G and d-reduction done per bank (3 wide op pairs),
                # then one indicator matmul accumulates b in ps_b.
                p4d = work.tile([128, GO], F32)
                for bk in range(3):
                    ps_g = ps_g_pool.tile([128, 3 * OD], F32)
                    for gi in range(3):
                        g = bk * 3 + gi
                        for m in range(M):
                            nc.tensor.matmul(
                                ps_g[:, gi * OD:(gi + 1) * OD],
                                xN[:, m * IL * J + g * 128:
                                   m * IL * J + (g + 1) * 128],
                                v_sb[:, m * OD:(m + 1) * OD],
                                start=(m == 0), stop=(m == M - 1))
                    p4 = work.tile([128, 3 * OD], F32, name=f"p4_{bk}")
                    nc.vector.tensor_tensor(
                        p4[:], w2[:, bk * 3 * OD:(bk + 1) * 3 * OD], ps_g[:],
                        op=Alu.mult)
                    nc.vector.reduce_sum(
                        p4d[:, bk * 30:(bk + 1) * 30],
                        p4[:].rearrange("p (g o d) -> p g o d", g=3, o=O),
                        axis=mybir.AxisListType.X)
                nc.tensor.matmul(ps_b, indj[:], p4d[:],
                                 start=True, stop=True)
                b_new = work.tile([16, GO], F32, name=f"b{t}")
                if t == 0:
                    nc.vector.tensor_copy(b_new[:], ps_b)
                else:
                    nc.vector.tensor_tensor(b_new[:], b_sb[:], ps_b,
                                            op=Alu.add)
                b_sb = b_new

    nc.compile()
    return nc


def _host_inputs(x, W):
    """Slice + lay out per-core inputs (bf16 matmul operands)."""
    import ml_dtypes
    bf16 = ml_dtypes.bfloat16
    x = np.ascontiguousarray(x, dtype=np.float32)
    W = np.ascontiguousarray(W, dtype=np.float32)
    ind = np.zeros((16, 128), dtype=np.float32)
    for k in range(16):
        ind[k, k * 8:(k + 1) * 8] = 1.0
    indj = np.ascontiguousarray(ind.T) / float(B)
    ind_bf = ind.astype(bf16)
    in_maps = []
    for c in range(N_CORES):
        sl = slice(c * IL, (c + 1) * IL)
        xs = x[:, sl, :].reshape(B, IL * J)
        ws = W[sl]  # [IL, O, D, J]
        in_maps.append({
            "xT": np.ascontiguousarray(xs.T).astype(bf16),
            "xN": np.ascontiguousarray(xs).astype(bf16),
            "w2": np.ascontiguousarray(
                ws.transpose(0, 3, 1, 2).reshape(IL * J, OD)).astype(bf16),
            "ind": ind_bf,
            "indj": indj,
        })
    return in_maps


def kernel(x, W):
    if "nc" not in _cache:
        _cache["nc"] = _build()
    nc = _cache["nc"]
    in_maps = _host_inputs(x, W)
    res = run_bass_kernel_spmd(nc, in_maps, list(range(N_CORES)))
    # ReduceScatter gave core k partitions [16k, 16k+16);
    # per-core output is [16, (m, od)] with global b = 128*m + 16*k + p
    v = np.empty((B, OD), dtype=np.float32)
    for k in range(N_CORES):
        vk = res.results[k]["v_out"].reshape(16, M, OD)
        for m in range(M):
            v[128 * m + 16 * k:128 * m + 16 * k + 16, :] = vk[:, m, :]
    return v.reshape(B, O, D, 1).astype(np.float32)
